# revision 10
# baseline (speedup 1.0000x reference)
"""ChannelAttentionPropagation1D kernel for 8x TRN2 NeuronCores.

Reference computation (per batch b):
  kv[c,d]   = sum_{t,n} key_mem[b,t,n,c] * val_mem[b,t,n,d]    # (64, 64)
  kv_soft   = softmax(kv, axis=c)
  out[n,d]  = alpha * (key_cur[b] @ kv_soft)[n,d] + val_cur[b,n,d]

Sharding (8 cores):
  phase 1: core i contracts the t=i slice of key_mem/val_mem (16384 tokens
           per batch) into a partial kv^T, then AllReduce (64 KB) over cores.
  phase 2: core i computes the n-slice [2048*i, 2048*(i+1)) of the output.

Layout notes:
  - phase 1 accumulates kvT[d,c] (PSUM) so the softmax axis c lands on the
    free axis; a tiny PE transpose afterwards yields kv_soft[c,d].
  - key_cur is transposed (and scaled by alpha) on the host so its channel
    axis is the SBUF partition axis; its token axis is permuted n = 16p + j
    so phase-2 output tiles assemble into 4KB-contiguous-per-partition
    stores.
"""

import numpy as np

import concourse.bacc as bacc
import concourse.mybir as mybir
import concourse.tile as tile
from concourse import bass_utils, masks

F32 = mybir.dt.float32

N_CORES = 8
N, T, NTOK, C, C2 = 4, 8, 16384, 64, 64
NSL = NTOK // N_CORES  # 2048: phase-2 token slice per core
A_TILES = 64           # 128-token matmul tiles per half-batch chunk
HALF = NTOK // 2       # 8192 tokens per phase-1 DMA chunk

_CACHE = {}

# Extra kwargs forwarded to run_bass_kernel_spmd (used by the profiling
# harness to request an NTFF trace; empty for normal correctness runs).
_RUN_OPTS = {}


def _build_program():
    nc = bacc.Bacc(
        "TRN2",
        target_bir_lowering=False,
        debug=False,
        enable_asserts=False,
        num_devices=N_CORES,
    )

    km = nc.dram_tensor("key_mem", [N, NTOK, C], F32, kind="ExternalInput").ap()
    vm = nc.dram_tensor("val_mem", [N, NTOK, C2], F32, kind="ExternalInput").ap()
    kct = nc.dram_tensor("key_curT", [N, C, NSL], F32, kind="ExternalInput").ap()
    vc = nc.dram_tensor("val_cur", [N, NSL, C2], F32, kind="ExternalInput").ap()
    out = nc.dram_tensor("out", [N, NSL, C2], F32, kind="ExternalOutput").ap()

    with tile.TileContext(nc) as tc:
        with (
            tc.tile_pool(name="persist", bufs=1) as persist,
            tc.tile_pool(name="big", bufs=3) as big,
            tc.tile_pool(name="tmp", bufs=2) as tmp,
            tc.tile_pool(name="stage", bufs=2) as stage_pool,
            tc.tile_pool(name="ps", bufs=2, space="PSUM") as ps,
            tc.tile_pool(name="dram", bufs=1, space="DRAM") as dram,
        ):
            ident = persist.tile([128, 128], F32)
            masks.make_identity(nc, ident[:])

            kct_sb = persist.tile([C, N * NSL], F32)
            vc_sb = persist.tile([128, N * (NSL // 128) * C2], F32)

            # Warmup collective: the first AllReduce on a fresh NEFF pays
            # ~20-26us (CC warmup + peer start skew). Absorb it here, fully
            # overlapped with phase-1 DMA/compute, so the real per-batch
            # AllReduces run at the ~10us floor.
            warm_in = dram.tile([1, 64], F32)
            warm_out = dram.tile([1, 64], F32, addr_space="Shared")
            warm_sb = persist.tile([1, 64], F32)
            nc.gpsimd.memset(warm_sb[:], 0.0)
            nc.scalar.dma_start(warm_in[:], warm_sb[:])
            nc.gpsimd.collective_compute(
                "AllReduce",
                mybir.AluOpType.add,
                replica_groups=[list(range(N_CORES))],
                ins=[warm_in.opt()],
                outs=[warm_out.opt()],
            )

            # ---- phase 1: partial kvT[d, c] per batch, col-tiled 2x ----
            # Even token-tiles accumulate on PE column group 0 (psum rows
            # 0:64), odd tiles on column group 2 (psum rows 64:128); the two
            # halves' LDWEIGHTS/MATMUL overlap on independent subarrays.
            kvt_sb = persist.tile([C2, N * C], F32)
            ar_ins, ar_outs = [], []
            for b in range(N):
                kv_ps = ps.tile([128, C], F32, tag="kv")
                for h in range(2):
                    k_sb = big.tile([128, HALF // 128 * C], F32, tag="k")
                    v_sb = big.tile([128, HALF // 128 * C2], F32, tag="v")
                    sl = slice(h * HALF, (h + 1) * HALF)
                    nc.sync.dma_start(
                        k_sb[:], km[b, sl, :].rearrange("(p a) c -> p (a c)", p=128)
                    )
                    nc.sync.dma_start(
                        v_sb[:], vm[b, sl, :].rearrange("(p a) c -> p (a c)", p=128)
                    )
                    if h == 1:
                        # phase-2 inputs for batch b: issued on the scalar
                        # (ACT) DMA FIFO so they never delay the phase-1
                        # chunk stream on the sync FIFO.
                        nc.scalar.dma_start(
                            kct_sb[:, b * NSL:(b + 1) * NSL], kct[b]
                        )
                        nc.scalar.dma_start(
                            vc_sb[:, b * 1024:(b + 1) * 1024],
                            vc[b].rearrange("(p j) c -> p (j c)", p=128),
                        )
                    for a in range(A_TILES):
                        half = a % 2
                        nc.tensor.matmul(
                            kv_ps[64 * half:64 * half + C2, :],
                            lhsT=v_sb[:, a * C2:(a + 1) * C2],
                            rhs=k_sb[:, a * C:(a + 1) * C],
                            start=(h == 0 and a < 2),
                            stop=(h == 1 and a >= A_TILES - 2),
                            tile_position=(0, 64 * half),
                        )
                # partial kvT = even-half + odd-half (DVE can read only one
                # PSUM operand per instruction, so copy then add)
                nc.vector.tensor_copy(kvt_sb[:, b * C:(b + 1) * C], kv_ps[0:C2, :])
                nc.vector.tensor_add(
                    kvt_sb[:, b * C:(b + 1) * C],
                    kvt_sb[:, b * C:(b + 1) * C],
                    kv_ps[64:64 + C2, :],
                )
                # per-batch AllReduce so later batches' compute hides it
                ar_in = dram.tile([C2, C], F32, tag=f"ar_in{b}")
                ar_out = dram.tile([C2, C], F32, addr_space="Shared", tag=f"ar_out{b}")
                nc.scalar.dma_start(ar_in[:], kvt_sb[:, b * C:(b + 1) * C])
                nc.gpsimd.collective_compute(
                    "AllReduce",
                    mybir.AluOpType.add,
                    replica_groups=[list(range(N_CORES))],
                    ins=[ar_in.opt()],
                    outs=[ar_out.opt()],
                )
                ar_ins.append(ar_in)
                ar_outs.append(ar_out)

            # ---- per batch: softmax + transpose + phase 2 + store ----
            kvt_red = persist.tile([C2, N * C], F32)
            kv_soft = persist.tile([C, N * C2], F32)
            for b in range(N):
                nc.scalar.dma_start(kvt_red[:, b * C:(b + 1) * C], ar_outs[b][:])
                s = kvt_red[:, b * C:(b + 1) * C]
                neg_mx = tmp.tile([C2, 1], F32, tag="mx")
                nc.vector.reduce_max(
                    out=neg_mx[:], in_=s, axis=mybir.AxisListType.X, negate=True
                )
                ex = tmp.tile([C2, C], F32, tag="ex")
                sm = tmp.tile([C2, 1], F32, tag="sm")
                nc.scalar.activation(
                    ex[:], s, mybir.ActivationFunctionType.Exp,
                    bias=neg_mx[:], scale=1.0, accum_out=sm[:],
                )
                rv = tmp.tile([C2, 1], F32, tag="rv")
                nc.vector.reciprocal(rv[:], sm[:])
                nc.vector.tensor_scalar_mul(ex[:], ex[:], rv[:])
                tp = ps.tile([C, C2], F32, tag="tp")
                nc.tensor.transpose(tp[:], ex[:], ident[0:C2, 0:C2])
                nc.vector.tensor_copy(kv_soft[:, b * C2:(b + 1) * C2], tp[:])

                stg = stage_pool.tile([128, (NSL // 128) * C2], F32, tag="stg")
                for j in range(NSL // 128):
                    o_ps = ps.tile([128, C2], F32, tag="o")
                    nc.tensor.matmul(
                        o_ps[:],
                        lhsT=kct_sb[:, b * NSL + j * 128: b * NSL + (j + 1) * 128],
                        rhs=kv_soft[:, b * C2:(b + 1) * C2],
                        start=True,
                        stop=True,
                    )
                    nc.vector.tensor_add(
                        stg[:, j * C2:(j + 1) * C2],
                        o_ps[:],
                        vc_sb[:, b * 1024 + j * C2: b * 1024 + (j + 1) * C2],
                    )
                nc.scalar.dma_start(
                    out[b].rearrange("(p j) c -> p (j c)", p=128), stg[:]
                )

    nc.compile()
    return nc


def _get_program():
    if "nc" not in _CACHE:
        _CACHE["nc"] = _build_program()
    return _CACHE["nc"]


def kernel(key_mem, val_mem, key_cur, val_cur, alpha):
    key_mem = np.asarray(key_mem, dtype=np.float32)
    val_mem = np.asarray(val_mem, dtype=np.float32)
    key_cur = np.asarray(key_cur, dtype=np.float32)
    val_cur = np.asarray(val_cur, dtype=np.float32)
    alpha_f = float(np.asarray(alpha).reshape(-1)[0])

    nc = _get_program()

    # key_cur^T with alpha folded in; token axis permuted so that SBUF
    # column j*128+p holds token p*16+j (phase-2 store contiguity).
    kc_scaled = (alpha_f * key_cur).astype(np.float32)
    in_maps = []
    for i in range(N_CORES):
        kct_i = kc_scaled[:, i * NSL:(i + 1) * NSL, :].transpose(0, 2, 1)
        kct_i = (
            kct_i.reshape(N, C, 128, NSL // 128)
            .transpose(0, 1, 3, 2)
            .reshape(N, C, NSL)
        )
        in_maps.append(
            {
                "key_mem": np.ascontiguousarray(key_mem[:, i]),
                "val_mem": np.ascontiguousarray(val_mem[:, i]),
                "key_curT": np.ascontiguousarray(kct_i),
                "val_cur": np.ascontiguousarray(val_cur[:, i * NSL:(i + 1) * NSL, :]),
            }
        )

    res = bass_utils.run_bass_kernel_spmd(
        nc, in_maps, core_ids=list(range(N_CORES)), **_RUN_OPTS
    )
    _CACHE["last_result"] = res
    outs = [res.results[i]["out"] for i in range(N_CORES)]
    return np.concatenate(outs, axis=1).astype(np.float32)
